# revision 13
# baseline (speedup 1.0000x reference)
"""Trainium2 Bass kernel for nn_CommCellInnerRNN (gnn_message_passing).

Strategy: data-parallel over the n_agents axis (4096 agents -> 8 cores x 512).
All on-device tensors use a [feature/unit (partition), agent (free)] layout so
the stacked-LSTM recurrence needs no transposes: weights are the stationary
matmul operand (lhsT), states are the moving operand (rhs), gates land in PSUM
in the same layout.

All matmuls run in fp16 (full PE rate, ~5e-4 rounding) with fp32 PSUM
accumulation; the LSTM c-state and gate math stay fp32.  h-states are kept in
fp16 (they are only ever matmul inputs).

Per neighbor-step t (17 steps: 16 neighbors + self):
  z1 = F + CWh1.T @ h1 + CWso.T @ augsig_t          (PSUM accumulation)
  gates = act(z1 + b)   (scalar engine, fused per-partition bias)
  c/h updates on DVE per 128-unit block (j-major) so the vector chain pipelines
  with the remaining matmuls; Keras mask semantics via copy_predicated.

The signal gather (agents may see agents on any shard) is resolved on the host:
the full [N, SIG] signal table is indexed with the (host-known, int32 input)
present_indices and shipped per-core as a [17, 261, 512] fp16 slab
(sig(256) + onehot dir(4) + dist(1), transposed).  F = CWx1_feat.T @ features
is precomputed on-device once and re-injected into each step's PSUM group via
an identity matmul.
"""

import os
import numpy as np

import concourse.bass as bass
from concourse import bacc
import concourse.mybir as mybir
import concourse.tile as tile
from concourse.bass_utils import run_bass_kernel_spmd

f32 = mybir.dt.float32
f16 = mybir.dt.float16

N, MAXO, IN_DIM = 4096, 16, 512
U1, U2 = 512, 512            # inner RNN units
CU1, CU2 = 512, 256          # comm RNN units
SIG = 256
T = MAXO + 1                 # neighbor steps + self
NCORES = 8
NS = N // NCORES             # 512 agents per core
P = 128

SIGF = mybir.ActivationFunctionType.Sigmoid
TANH = mybir.ActivationFunctionType.Tanh
COPYF = mybir.ActivationFunctionType.Copy
GFUNC = [SIGF, SIGF, TANH, SIGF]   # i, f, g, o


def _ts(i, n):
    return slice(i * n, (i + 1) * n)


def build_nc(t_steps=T):
    nc = bacc.Bacc("TRN2", debug=False, num_devices=NCORES)

    def inp(name, shape, dtype=f32):
        return nc.dram_tensor(name, list(shape), dtype, kind="ExternalInput").ap()

    xT = inp("xT", (P, 4 * NS), f16)
    rh1 = inp("rh1", (P, 4 * NS), f16)
    rc1 = inp("rc1", (P, 4 * NS))
    rh2 = inp("rh2", (P, 4 * NS), f16)
    rc2 = inp("rc2", (P, 4 * NS))
    ch1_i = inp("ch1", (P, 4 * NS), f16)
    cc1_i = inp("cc1", (P, 4 * NS))
    ch2_i = inp("ch2", (P, 2 * NS), f16)
    cc2_i = inp("cc2", (P, 2 * NS))

    Wx1 = inp("Wx1", (P, 4 * 2048), f16)     # [512,2048] -> 4 K-blocks [128,2048]
    Wh1 = inp("Wh1", (P, 4 * 2048), f16)
    Wx2 = inp("Wx2", (P, 4 * 2048), f16)
    Wh2 = inp("Wh2", (P, 4 * 2048), f16)
    CWh1 = inp("CWh1", (P, 4 * 2048), f16)
    CWx1f = inp("CWx1f", (P, 4 * 2048), f16)
    CWso01 = inp("CWso01", (P, 2 * 2048), f16)   # rows 0:256 of CWx1
    CWso2 = inp("CWso2", (5, 2048), f16)         # rows 256:261 of CWx1
    CWx2 = inp("CWx2", (P, 4 * 1024), f16)   # [512,1024] -> 4 K-blocks [128,1024]
    CWh2 = inp("CWh2", (P, 2 * 1024), f16)   # [256,1024] -> 2 K-blocks

    b1 = inp("b1", (P, 16))
    b2 = inp("b2", (P, 16))
    Cb1 = inp("Cb1", (P, 16))
    Cb2 = inp("Cb2", (P, 8))
    ident = inp("ident", (P, P), f16)

    augsig = inp("augsig", (T, 261, NS), f16)
    masku = inp("masku", (T, P, NS), mybir.dt.uint8)  # broadcast step masks

    osig = nc.dram_tensor("osig", [P, 2 * NS], f16, kind="ExternalOutput").ap()
    feat_o = nc.dram_tensor("feat", [P, 4 * NS], f32, kind="ExternalOutput").ap()

    with tile.TileContext(nc) as tc:
        with (
            tc.tile_pool(name="pers", bufs=1) as pers,
            tc.tile_pool(name="wstream", bufs=8) as wstream,
            tc.tile_pool(name="sstream", bufs=3) as sstream,
            tc.tile_pool(name="mstream", bufs=3) as mstream,
            tc.tile_pool(name="psum", bufs=8, space="PSUM") as psum,
        ):
            # ---------- persistent SBUF ----------
            t_cwh1 = pers.tile([P, 4 * 2048], f16, tag="cwh1")
            nc.sync.dma_start(t_cwh1[:], CWh1[:])
            t_cwso01 = pers.tile([P, 2 * 2048], f16, tag="cwso01")
            nc.sync.dma_start(t_cwso01[:], CWso01[:])
            t_cwso2 = pers.tile([P, 2048], f16, tag="cwso2")
            nc.sync.dma_start(t_cwso2[:5, :], CWso2[:])
            t_cwx2 = pers.tile([P, 4 * 1024], f16, tag="cwx2")
            nc.sync.dma_start(t_cwx2[:], CWx2[:])
            t_cwh2 = pers.tile([P, 2 * 1024], f16, tag="cwh2")
            nc.sync.dma_start(t_cwh2[:], CWh2[:])
            t_b1 = pers.tile([P, 16], f32, tag="b1")
            nc.sync.dma_start(t_b1[:], b1[:])
            t_b2 = pers.tile([P, 16], f32, tag="b2")
            nc.sync.dma_start(t_b2[:], b2[:])
            t_cb1 = pers.tile([P, 16], f32, tag="cb1")
            nc.sync.dma_start(t_cb1[:], Cb1[:])
            t_cb2 = pers.tile([P, 8], f32, tag="cb2")
            nc.sync.dma_start(t_cb2[:], Cb2[:])
            t_ident = pers.tile([P, P], f16, tag="ident")
            nc.sync.dma_start(t_ident[:], ident[:])

            t_z1 = pers.tile([P, 16 * NS], f32, tag="z1")       # gate scratch
            t_feat = pers.tile([P, 4 * NS], f32, tag="feat")    # features^T out
            t_feat16 = pers.tile([P, 4 * NS], f16, tag="feat16")
            t_F = pers.tile([P, 16 * NS], f16, tag="F")
            t_h1n = pers.tile([P, 4 * NS], f16, tag="h1n")      # raw new h1
            t_h2n = pers.tile([P, 2 * NS], f16, tag="h2n")      # raw new h2

            # phase-0 state tiles; slots recycled for comm states later
            t_x = pers.tile([P, 4 * NS], f16, tag="sA")
            nc.sync.dma_start(t_x[:], xT[:])
            t_rh1 = pers.tile([P, 4 * NS], f16, tag="sB")
            nc.sync.dma_start(t_rh1[:], rh1[:])
            t_rc1 = pers.tile([P, 4 * NS], f32, tag="sC")
            nc.sync.dma_start(t_rc1[:], rc1[:])
            t_rh2 = pers.tile([P, 4 * NS], f16, tag="sD")
            nc.sync.dma_start(t_rh2[:], rh2[:])
            t_rc2 = pers.tile([P, 4 * NS], f32, tag="sE")
            nc.sync.dma_start(t_rc2[:], rc2[:])

            zi = t_z1[:, 0 * 4 * NS:1 * 4 * NS]
            zf = t_z1[:, 1 * 4 * NS:2 * 4 * NS]
            zg = t_z1[:, 2 * 4 * NS:3 * 4 * NS]
            zo = t_z1[:, 3 * 4 * NS:4 * 4 * NS]

            # ================= phase 0: inner stacked LSTM =================
            # L1: z = Wx1.T @ x + Wh1.T @ h; weights streamed once (fp16).
            wt1 = []
            for k in range(8):
                w = wstream.tile([P, 2048], f16, tag="w", name="w")
                src = Wx1 if k < 4 else Wh1
                nc.sync.dma_start(w[:], src[:, _ts(k % 4, 2048)])
                wt1.append(w)
            for half in range(2):
                pts = [psum.tile([P, NS], f32, tag="pt", name="pt") for _ in range(8)]
                for k in range(8):
                    rhs = (t_x if k < 4 else t_rh1)[:, _ts(k % 4, NS)]
                    for mi in range(8):
                        m = half * 8 + mi
                        nc.tensor.matmul(
                            pts[mi][:], wt1[k][:, _ts(m, P)], rhs,
                            start=(k == 0), stop=(k == 7),
                        )
                for mi in range(8):
                    m = half * 8 + mi
                    nc.scalar.activation(
                        t_z1[:, _ts(m, NS)], pts[mi][:], GFUNC[m // 4],
                        bias=t_b1[:, m:m + 1],
                    )

            nc.vector.tensor_mul(zi, zi, zg)          # sig(i)*tanh(g)
            nc.vector.tensor_mul(zg, zf, t_rc1[:])    # sig(f)*c
            nc.vector.tensor_add(zg, zg, zi)          # c1'
            nc.scalar.activation(zf, zg, TANH)
            nc.vector.tensor_mul(t_h1n[:], zo, zf)    # h1' (fp16, L2 input)

            # L2: z = Wx2.T @ h1' + Wh2.T @ rh2, two unit-block halves through
            # z1[:, 0:4096] scratch (i/f regions, dead by now).
            wt2 = []
            for k in range(8):
                w = wstream.tile([P, 2048], f16, tag="w", name="w")
                src = Wx2 if k < 4 else Wh2
                nc.sync.dma_start(w[:], src[:, _ts(k % 4, 2048)])
                wt2.append(w)
            for half in range(2):
                zz = t_z1[:, 0:8 * NS]
                pts = [psum.tile([P, NS], f32, tag="pt", name="pt") for _ in range(8)]
                mlist = [(X, jb) for X in range(4) for jb in range(2)]
                for k in range(8):
                    rhs = (t_h1n if k < 4 else t_rh2)[:, _ts(k % 4, NS)]
                    for mi, (X, jb) in enumerate(mlist):
                        m = 4 * X + 2 * half + jb
                        nc.tensor.matmul(
                            pts[mi][:], wt2[k][:, _ts(m, P)], rhs,
                            start=(k == 0), stop=(k == 7),
                        )
                for mi, (X, jb) in enumerate(mlist):
                    m = 4 * X + 2 * half + jb
                    nc.scalar.activation(
                        zz[:, _ts(2 * X + jb, NS)], pts[mi][:], GFUNC[X],
                        bias=t_b2[:, m:m + 1],
                    )
                z2i = zz[:, 0 * 2 * NS:1 * 2 * NS]
                z2f = zz[:, 1 * 2 * NS:2 * 2 * NS]
                z2g = zz[:, 2 * 2 * NS:3 * 2 * NS]
                z2o = zz[:, 3 * 2 * NS:4 * 2 * NS]
                rc2h = t_rc2[:, half * 2 * NS:(half + 1) * 2 * NS]
                nc.vector.tensor_mul(z2i, z2i, z2g)
                nc.vector.tensor_mul(z2g, z2f, rc2h)
                nc.vector.tensor_add(z2g, z2g, z2i)
                nc.scalar.activation(z2f, z2g, TANH)
                nc.vector.tensor_mul(z2o, z2o, z2f)   # h2' half (features)
                hs = slice(half * 2 * NS, (half + 1) * 2 * NS)
                nc.vector.tensor_copy(t_feat[:, hs], z2o)
                nc.scalar.activation(t_feat16[:, hs], z2o, COPYF)

            nc.sync.dma_start(feat_o[:], t_feat[:])

            # F = CWx1_feat.T @ features
            wtf = []
            for k in range(4):
                w = wstream.tile([P, 2048], f16, tag="w", name="w")
                nc.sync.dma_start(w[:], CWx1f[:, _ts(k, 2048)])
                wtf.append(w)
            for half in range(2):
                pts = [psum.tile([P, NS], f32, tag="pt", name="pt") for _ in range(8)]
                for k in range(4):
                    rhs = t_feat16[:, _ts(k, NS)]
                    for mi in range(8):
                        m = half * 8 + mi
                        nc.tensor.matmul(
                            pts[mi][:], wtf[k][:, _ts(m, P)], rhs,
                            start=(k == 0), stop=(k == 3),
                        )
                for mi in range(8):
                    m = half * 8 + mi
                    nc.scalar.activation(t_F[:, _ts(m, NS)], pts[mi][:], COPYF)

            # comm states into recycled slots
            t_ch1 = pers.tile([P, 4 * NS], f16, tag="sA")
            nc.sync.dma_start(t_ch1[:], ch1_i[:])
            t_cc1 = pers.tile([P, 4 * NS], f32, tag="sB")
            nc.sync.dma_start(t_cc1[:], cc1_i[:])
            t_ch2 = pers.tile([P, 2 * NS], f16, tag="sC")
            nc.sync.dma_start(t_ch2[:], ch2_i[:])
            t_cc2 = pers.tile([P, 2 * NS], f32, tag="sD")
            nc.sync.dma_start(t_cc2[:], cc2_i[:])

            # ================= phase 1: comm LSTM over 17 neighbor steps ====
            for t in range(t_steps):
                a0 = sstream.tile([P, NS], f16, tag="a0", name="a0")
                nc.sync.dma_start(a0[:], augsig[t, 0:128, :])
                a1 = sstream.tile([P, NS], f16, tag="a1", name="a1")
                nc.sync.dma_start(a1[:], augsig[t, 128:256, :])
                a2 = sstream.tile([P, NS], f16, tag="a2", name="a2")
                nc.sync.dma_start(a2[:5, :], augsig[t, 256:261, :])
                mk = mstream.tile([P, NS], mybir.dt.uint8, tag="mk", name="mk")
                nc.sync.dma_start(mk[:], masku[t])

                # ---- comm L1: j-major so the DVE chain pipelines with PE ----
                for j in range(4):
                    for g in range(4):
                        m = 4 * g + j
                        pt = psum.tile([P, NS], f32, tag="pt", name="pt")
                        nc.tensor.matmul(pt[:], t_ident[:], t_F[:, _ts(m, NS)],
                                         start=True, stop=False)
                        for k in range(4):
                            nc.tensor.matmul(
                                pt[:],
                                t_cwh1[:, k * 2048 + m * P:k * 2048 + (m + 1) * P],
                                t_ch1[:, _ts(k, NS)], start=False, stop=False,
                            )
                        nc.tensor.matmul(pt[:], t_cwso01[:, _ts(m, P)], a0[:],
                                         start=False, stop=False)
                        nc.tensor.matmul(
                            pt[:], t_cwso01[:, 2048 + m * P:2048 + (m + 1) * P],
                            a1[:], start=False, stop=False)
                        nc.tensor.matmul(pt[:], t_cwso2[:5, _ts(m, P)], a2[:5, :],
                                         start=False, stop=True)
                        nc.scalar.activation(
                            t_z1[:, _ts(m, NS)], pt[:], GFUNC[g],
                            bias=t_cb1[:, m:m + 1],
                        )
                    # unit-block j chain
                    js = _ts(j, NS)
                    zi_j, zf_j = zi[:, js], zf[:, js]
                    zg_j, zo_j = zg[:, js], zo[:, js]
                    nc.vector.tensor_mul(zi_j, zi_j, zg_j)
                    nc.vector.tensor_mul(zg_j, zf_j, t_cc1[:, js])
                    nc.vector.tensor_add(zg_j, zg_j, zi_j)       # c1_new
                    nc.scalar.activation(zf_j, zg_j, TANH)
                    nc.vector.tensor_mul(t_h1n[:, js], zo_j, zf_j)
                    nc.vector.copy_predicated(t_cc1[:, js], mk[:], zg_j)

                # ch1 is a K-input of every L1 m-tile above, so its update
                # must come after the whole m-loop (unlike the per-block cc1).
                for j in range(4):
                    js = _ts(j, NS)
                    nc.vector.copy_predicated(t_ch1[:, js], mk[:], t_h1n[:, js])

                # ---- comm L2 (K-outer over h1_new blocks) ----
                zz = t_z1[:, 0:8 * NS]
                pts = [psum.tile([P, NS], f32, tag="pt", name="pt") for _ in range(8)]
                for k in range(4):
                    for m in range(8):
                        nc.tensor.matmul(
                            pts[m][:],
                            t_cwx2[:, k * 1024 + m * P:k * 1024 + (m + 1) * P],
                            t_h1n[:, _ts(k, NS)], start=(k == 0), stop=False,
                        )
                for k in range(2):
                    for m in range(8):
                        nc.tensor.matmul(
                            pts[m][:],
                            t_cwh2[:, k * 1024 + m * P:k * 1024 + (m + 1) * P],
                            t_ch2[:, _ts(k, NS)], start=False, stop=(k == 1),
                        )
                for jj in range(2):
                    for g in range(4):
                        m = 2 * g + jj
                        nc.scalar.activation(
                            zz[:, _ts(m, NS)], pts[m][:], GFUNC[g],
                            bias=t_cb2[:, m:m + 1],
                        )
                    jjs = _ts(jj, NS)
                    z2i = zz[:, 0 * 2 * NS:1 * 2 * NS][:, jjs]
                    z2f = zz[:, 1 * 2 * NS:2 * 2 * NS][:, jjs]
                    z2g = zz[:, 2 * 2 * NS:3 * 2 * NS][:, jjs]
                    z2o = zz[:, 3 * 2 * NS:4 * 2 * NS][:, jjs]
                    nc.vector.tensor_mul(z2i, z2i, z2g)
                    nc.vector.tensor_mul(z2g, z2f, t_cc2[:, jjs])
                    nc.vector.tensor_add(z2g, z2g, z2i)          # c2_new
                    nc.scalar.activation(z2f, z2g, TANH)
                    nc.vector.tensor_mul(t_h2n[:, jjs], z2o, z2f)
                    nc.vector.copy_predicated(t_cc2[:, jjs], mk[:], z2g)
                    nc.vector.copy_predicated(t_ch2[:, jjs], mk[:], t_h2n[:, jjs])

            nc.sync.dma_start(osig[:], t_ch2[:])

    nc.compile()
    return nc


# ---------------------------------------------------------------------------
# host-side prep
# ---------------------------------------------------------------------------

def _blk(a):
    """[R, C] (R = rt*128) -> [128, rt*C] with block k = rows k*128:(k+1)*128."""
    R, C = a.shape
    rt = R // P
    return np.ascontiguousarray(
        a.reshape(rt, P, C).transpose(1, 0, 2).reshape(P, rt * C))


def _unblk(b, R):
    Pp, X = b.shape
    rt = R // P
    C = X // rt
    return np.ascontiguousarray(
        b.reshape(P, rt, C).transpose(1, 0, 2).reshape(R, C))


def _bias_blk(b):
    """[4U] -> [128, 4U/128] column per m-tile."""
    return np.ascontiguousarray(b.reshape(-1, P).T)


_NC_CACHE = {}
LAST_EXEC_NS = {}


def prep_in_maps(inputs):
    inputs = {k: np.asarray(v) for k, v in inputs.items()}
    f = lambda k: inputs[k].astype(np.float32)
    h = np.float16

    shared = {
        "Wx1": _blk(f("Wx1")).astype(h), "Wh1": _blk(f("Wh1")).astype(h),
        "Wx2": _blk(f("Wx2")).astype(h), "Wh2": _blk(f("Wh2")).astype(h),
        "CWh1": _blk(f("CWh1")).astype(h),
        "CWx1f": _blk(f("CWx1")[SIG + 5:]).astype(h),
        "CWso01": _blk(f("CWx1")[0:SIG]).astype(h),
        "CWso2": np.ascontiguousarray(f("CWx1")[SIG:SIG + 5]).astype(h),
        "CWx2": _blk(f("CWx2")).astype(h), "CWh2": _blk(f("CWh2")).astype(h),
        "b1": _bias_blk(f("b1")), "b2": _bias_blk(f("b2")),
        "Cb1": _bias_blk(f("Cb1")), "Cb2": _bias_blk(f("Cb2")),
        "ident": np.eye(P, dtype=h),
    }

    signals = f("signals")
    padded_signals = np.concatenate([np.zeros((1, SIG), np.float32), signals], 0)
    pidx = inputs["present_indices"]
    ids_all = pidx[:, 0::3]
    dists_all = pidx[:, 1::3].astype(np.float32)
    dirs_all = pidx[:, 2::3]

    in_maps = []
    for c in range(NCORES):
        s = slice(c * NS, (c + 1) * NS)
        m = dict(shared)
        m["xT"] = _blk(f("inputs")[s].T).astype(h)
        m["rh1"] = _blk(f("rnn_h1")[s].T).astype(h)
        m["rc1"] = _blk(f("rnn_c1")[s].T)
        m["rh2"] = _blk(f("rnn_h2")[s].T).astype(h)
        m["rc2"] = _blk(f("rnn_c2")[s].T)
        m["ch1"] = _blk(f("comm_h1")[s].T).astype(h)
        m["cc1"] = _blk(f("comm_c1")[s].T)
        m["ch2"] = _blk(f("comm_h2")[s].T).astype(h)
        m["cc2"] = _blk(f("comm_c2")[s].T)

        ids = ids_all[s]
        dists = dists_all[s]
        dirs = dirs_all[s]
        aug = np.zeros((T, 261, NS), np.float32)
        mkv = np.zeros((T, NS), np.float32)
        for t in range(MAXO):
            aug[t, 0:SIG] = padded_signals[ids[:, t] + 1].T
            oh = np.zeros((NS, 4), np.float32)
            oh[np.arange(NS), dirs[:, t]] = 1.0
            aug[t, SIG:SIG + 4] = oh.T
            aug[t, SIG + 4] = dists[:, t]
            mkv[t] = (ids[:, t] >= 0).astype(np.float32)
        self_ids = np.arange(c * NS, (c + 1) * NS)
        aug[MAXO, 0:SIG] = signals[self_ids].T
        mkv[MAXO] = 1.0
        m["augsig"] = aug.astype(h)
        m["masku"] = np.ascontiguousarray(
            np.broadcast_to(mkv[:, None, :], (T, P, NS))).astype(np.uint8)
        in_maps.append(m)
    return in_maps


def kernel(**inputs):
    in_maps = prep_in_maps(inputs)

    if "nc" not in _NC_CACHE:
        _NC_CACHE["nc"] = build_nc()
    nc = _NC_CACHE["nc"]

    trace = bool(int(os.environ.get("KERNEL_TRACE", "0")))
    res = run_bass_kernel_spmd(
        nc, in_maps, core_ids=list(range(NCORES)), trace=trace)
    LAST_EXEC_NS["exec_time_ns"] = res.exec_time_ns
    LAST_EXEC_NS["res"] = res

    outs = []
    for c in range(NCORES):
        osig = _unblk(res.results[c]["osig"].astype(np.float32), CU2)
        feat = _unblk(res.results[c]["feat"], U2)
        outs.append(np.concatenate([osig.T, osig.T, feat.T], axis=1))
    return np.concatenate(outs, axis=0).astype(np.float32)
